# revision 28
# baseline (speedup 1.0000x reference)
"""KANLinear forward on 8 Trainium2 NeuronCores (data-parallel over batch).

Factorization (v2: 7 K-blocks instead of 8)
-------------------------------------------
reference computes, per token row x (after clip/renorm preprocessing):
    y = silu(x) @ base_weight.T + einsum('big,oig->bo', bsplines(x), sw*scaler)

With s = 2.5*x + 5.5 the 8 cubic B-spline bases are B_g(x) = N3(s-g),
g = 0..7.  The silu path folds into the same basis (least-squares
projection), giving per (i,o) a weight vector W_g over the 8 bases.

v2 change-of-basis: interpolate a cubic p(g) = sum_m c_m*T_m(g),
T_m(g) = ((g-3.5)/2)^m, through W at g in {0,2,5,7}; the residual
r_g = W_g - p(g) is nonzero only at g in {1,3,4,6}.  Moment identities
for cubic B-splines (tau := (s-5.5)/2):
    sum_g T1(g) N3(s-g) = tau
    sum_g T2(g) N3(s-g) = tau^2 + 1/12
    sum_g T3(g) N3(s-g) = tau^3 + tau/4
so the polynomial part rides on cheap monomial features {tau, tau^2,
tau^3}, the constant parts collapse into a per-output bias added at
PSUM drain, and only 4 spline bases remain as ACT-table evaluations.
K shrinks 4096 -> 3584 (8 blocks -> 7) and the expensive per-element
spline evals drop 8 -> 4.  Edge defects (the g-sum truncation at
s<3 / s>8, where phantom bases g=-1/g=8 would be needed) contribute
rel err ~5e-3 (verified vs reference in fp64), within the 2e-2 gate.

Features 6*N3(s-g) are produced by ScalarE ACTIVATE through a custom
ACT table (the stock `sin` entry rewritten so activation(Sin,
scale=0.25, bias=(9.5-g)/8) returns 6*N3(s-g) exactly); tau powers by
two VectorE tensor_tensor ops off the KAN_PRE-preprocessed input.
Batch dim (16384) is sharded 2048 rows/core; weights are replicated.
"""

import hashlib
import os
import shutil
import tempfile

import numpy as np

B, IN_F, OUT_F = 16384, 512, 512
N_CORES = 8
BPC = B // N_CORES            # batch rows per core
BS = 512                      # batch-column slice processed per step
N_BS = BPC // BS              # 4 slices
N_IT = IN_F // 128            # 4 input-feature partition tiles
NB = 7                        # K-blocks per input tile (3 monomial + 4 spline)
KC = N_IT * NB                # 28 K-chunks of 128
G_INT = (0, 2, 5, 7)          # interpolation nodes (weights exactly absorbed)
G_RES = (1, 3, 4, 6)          # residual spline bases kept as ACT features
N_WARM_MM = 7                 # dummy matmuls to ramp the PE clock at startup

_state = {}


# --------------------------------------------------------------------------
# Custom ACT table: hijack `sin` in silu_and_others to evaluate 6*N3(8u-4).
# Verified-on-HW stock mapping: ctrl entry = 42+(exp-116); entry 52 (binade
# [0.5,1)) has 8 sub-buckets of width 1/16 at buckets 1034..1041; bucket
# eval is y = d0+(u-x0)(d1+(u-x0)(d2+(u-x0)d3)); |u|<2^-11 -> bucket
# 1075/1076 (sign-folded); large |u| -> 1077/1078.  Buckets 1020..1078 are
# sin-private; everything else (silu, copy, ...) is untouched.
# --------------------------------------------------------------------------
def _n3_6_coeffs(j):
    return {
        0: [0.0, 0.0, 0.0, 1.0],
        1: [1.0, 3.0, 3.0, -3.0],
        2: [4.0, 0.0, -6.0, 3.0],
        3: [1.0, -3.0, 3.0, -1.0],
    }[j]


def _compose(c, scale, shift):
    c0, c1, c2, c3 = c
    return [
        c0 + c1 * shift + c2 * shift**2 + c3 * shift**3,
        scale * (c1 + 2 * c2 * shift + 3 * c3 * shift**2),
        scale**2 * (c2 + 3 * c3 * shift),
        scale**3 * c3,
    ]


def _build_custom_act_root():
    if "act_root" in _state:
        return _state["act_root"], _state["act_sig"]
    from neuronxcc.driver.Job import Job
    from neuronxcc.driver.jobs.support.FindActInfo import findActInfoFile

    src_json = findActInfoFile(Job.getPackageDir(), "gen3")
    src_dir = os.path.dirname(src_json)
    dst_dir = tempfile.mkdtemp(prefix="kan_act_root_")
    for f in os.listdir(src_dir):
        shutil.copy(os.path.join(src_dir, f), os.path.join(dst_dir, f))
    for f in os.listdir(dst_dir):
        os.chmod(os.path.join(dst_dir, f), 0o644)

    bkt_path = os.path.join(dst_dir, "silu_and_others_bkt.bin")
    bkt = np.fromfile(bkt_path, dtype=np.float32).reshape(-1, 8).copy()
    bkt[1020:1079] = 0.0
    for k in range(8):
        x0 = 0.5 + k / 16.0 + 1.0 / 32.0
        j = k // 2
        q = _compose(_n3_6_coeffs(j), 8.0, 8.0 * x0 - 4.0 - j)
        bkt[1034 + k] = [q[0], q[1], q[2], q[3], x0, 0.0, 0.0, 0.0]
    bkt.tofile(bkt_path)

    sig = hashlib.sha256(open(bkt_path, "rb").read()).hexdigest()[:10]
    path = os.path.join(dst_dir, "act_info.json")
    os.environ["BASS_ACT_ROOT_JSON_PATH"] = path
    _state["act_root"] = path
    _state["act_sig"] = sig
    return path, sig


# --------------------------------------------------------------------------
# Custom DVE op: preprocessing clip(x,-1.1,1.1)*1.25 -> tau = (s-5.5)/2
# --------------------------------------------------------------------------
def _register_ops():
    if "ops" in _state:
        return _state["ops"]
    import concourse.dve_ops as dve_ops
    from concourse.dve_spec import Spec, Src0, C0, C1, C2, One, maxx, minn, lower
    from concourse.dve_uop import DveOpSpec

    def pre_ref(in0, in1, s0, s1, imm2):
        t = np.minimum(np.maximum(in0, np.float32(s0)), np.float32(s1))
        t = ((t + np.float32(1)) - np.float32(1)).astype(np.float32)
        return (t * np.float32(imm2)).astype(np.float32)

    pre_spec = Spec(
        body=((minn(maxx(Src0, C0), C1) + One) - One) * C2, reference=pre_ref
    )

    ops = {}
    name = "KAN_PRE"
    if name in dve_ops._SUB_OPCODE_FOR_NAME:
        ops[name] = next(o for o in dve_ops.OPS if o.name == name)
    else:
        row = dve_ops._CUSTOM_DVE_ROW_BASE + len(dve_ops.OPS)
        assert row < 0x20, "custom-DVE row overflow"
        shas = {}
        for ver in ("v3", "v4"):
            try:
                tmp = DveOpSpec(
                    name=name, opcode=row, uops=lower(pre_spec, ver=ver),
                    rd1_en=dve_ops.has_src1(pre_spec),
                )
                shas[ver] = tmp.sha(ver)
            except Exception:
                pass
        op = dve_ops.DveOp(name, pre_spec, subdim=False, uops_sha=shas)
        dve_ops.OPS.append(op)
        dve_ops._SUB_OPCODE_FOR_NAME[name] = row
        dve_ops.CUSTOM_DVE_SPECS[name] = pre_spec
        ops[name] = op
    _state["ops"] = ops
    return ops


# --------------------------------------------------------------------------
# Kernel build
# --------------------------------------------------------------------------
def _build_kernel():
    if "nc" in _state:
        return _state["nc"]
    import concourse.bacc as bacc
    import concourse.mybir as mybir
    import concourse.tile as tile
    from concourse.bass import ts

    _, act_sig = _build_custom_act_root()
    ops = _register_ops()
    f32 = mybir.dt.float32
    bf16 = mybir.dt.bfloat16
    AF = mybir.ActivationFunctionType
    ALU = mybir.AluOpType

    nc = bacc.Bacc()
    # Register const APs for the per-basis ACT biases.  The act-table
    # signature is baked into the tensor name so NEFF caches can never mix
    # incompatible act tables with this BIR.
    for g in G_RES:
        val = (9.5 - g) / 8.0
        t = nc.alloc_sbuf_tensor(f"cbias{g}-{act_sig}", [128, 1], f32)
        nc.gpsimd.memset(t.ap(), val)
        nc.const_aps.aps[(f32, val)] = t.ap()
    nc.all_engine_barrier()

    xT = nc.dram_tensor("xT", [IN_F, BPC], f32, kind="ExternalInput")
    # V is laid out partition-major on the host ([sbuf partition, chunk, out])
    # so each per-partition DMA run is one contiguous 28KB read.
    V = nc.dram_tensor("V", [128, KC * OUT_F], bf16, kind="ExternalInput")
    # host-permuted: bvec[p, t] = bias[t*128 + p] so the DMA is contiguous
    bvec = nc.dram_tensor("bvec", [128, N_IT], f32, kind="ExternalInput")
    yT = nc.dram_tensor("yT", [OUT_F, BPC], f32, kind="ExternalOutput")

    with tile.TileContext(nc) as tc:
        with (
            tc.tile_pool(name="vpool", bufs=1) as vpool,
            tc.tile_pool(name="warmp", bufs=1) as warm_pool,
            tc.tile_pool(name="xin", bufs=6) as xin_pool,
            tc.tile_pool(name="xs", bufs=4) as xs_pool,
            tc.tile_pool(name="feat", bufs=8) as feat_pool,
            tc.tile_pool(name="ysb", bufs=6) as ysb_pool,
            tc.tile_pool(name="psum", bufs=8, space="PSUM") as psum_pool,
        ):
            # Weight DMAs first on the gpsimd queue so the transfers start
            # the moment the engine exits the NEFF preamble.
            v_sb = vpool.tile([128, KC, OUT_F], bf16)
            v_view = V[:].rearrange("p (kc o) -> p kc o", o=OUT_F)
            # it0 alone first so its completion semaphore posts the moment it
            # streams (swdge finishes a trigger fully before the next one);
            # the remaining tiles ride one big second trigger.
            # it0 arrives in two pieces: the monomial planes (0-2) land first
            # so the earliest ci-major matmuls can start ~1.5us sooner.
            nc.sync.dma_start(v_sb[:, 0:3, :], v_view[:, 0:3, :])
            nc.gpsimd.dma_start(v_sb[:, 3:NB, :], v_view[:, 3:NB, :])
            bias_sb = vpool.tile([128, N_IT], f32)
            nc.gpsimd.dma_start(bias_sb[:], bvec[:])
            # bs0's x tiles are queued on sync BEFORE the remaining weight
            # tiles so the first feature chain isn't starved for bandwidth;
            # V-it1..3 follow on the same ring and still land with margin.
            xins0 = []
            for it in range(N_IT):
                xin = xin_pool.tile([128, BS], f32)
                nc.sync.dma_start(xin[:], xT[ts(it, 128), 0:BS])
                xins0.append(xin)
            for it in range(1, N_IT):
                nc.sync.dma_start(
                    v_sb[:, ts(it, NB), :], v_view[:, ts(it, NB), :]
                )

            # Kick the ACT table load for silu_and_others immediately so it
            # overlaps the first input DMA instead of the first feature chain.
            warm = xs_pool.tile([128, 1], f32, name="warm", tag="warm")
            nc.vector.memset(warm[:], 0.0)
            nc.scalar.activation(warm[:], warm[:], AF.Silu)

            # PE clock-ramp dummies: zero matmuls with no DMA dependency run
            # while the weight/input DMAs are in flight, so the real matmuls
            # start at full clock.
            wstat = warm_pool.tile([128, 128], bf16, name="wstat")
            wmov = warm_pool.tile([128, BS], bf16, name="wmov")
            nc.vector.memset(wstat[:], 0.0)
            nc.vector.memset(wmov[:], 0.0)
            wacc = psum_pool.tile([128, BS], f32, name="wacc", tag="acc")
            for _ in range(N_WARM_MM):
                nc.tensor.matmul(wacc[:], wstat[:], wmov[:], start=True, stop=True)



            for bs in range(N_BS):
                accs = [
                    psum_pool.tile([128, BS], f32, name=f"acc{o}", tag="acc")
                    for o in range(N_IT)
                ]
                for it in range(N_IT):
                    if bs == 0:
                        xin = xins0[it]
                    else:
                        xin = xin_pool.tile([128, BS], f32)
                        nc.sync.dma_start(xin[:], xT[ts(it, 128), ts(bs, BS)])
                    # tau = clip(x,-1.1,1.1)*1.25 = (s-5.5)/2, fp32
                    xs = xs_pool.tile([128, BS], f32)
                    nc.vector._custom_dve(
                        ops["KAN_PRE"], out=xs[:], in0=xin[:],
                        s0=-1.1, s1=1.1, imm2=1.25,
                    )
                    ft = feat_pool.tile([128, NB, BS], bf16)
                    # monomial features: tau, tau^2, tau^3 (VectorE)
                    nc.vector.tensor_scalar_mul(ft[:, 0, :], xs[:], 1.0)
                    nc.vector.tensor_tensor(
                        ft[:, 1, :], xs[:], xs[:], ALU.mult
                    )
                    nc.vector.tensor_tensor(
                        ft[:, 2, :], ft[:, 1, :], xs[:], ALU.mult
                    )
                    # residual spline bases 6*N3(s-g) via custom ACT table
                    for k, g in enumerate(G_RES):
                        nc.scalar.activation(
                            ft[:, 3 + k, :], xs[:], AF.Sin,
                            scale=0.25, bias=(9.5 - g) / 8.0,
                        )
                    # ci-major on the very first tile so the earliest matmuls
                    # depend only on the fast DVE monomial planes while the
                    # ACT-table bases are still loading; o-major elsewhere so
                    # drains stagger at the end of each batch slice.
                    if bs == 0 and it == 0:
                        order = [(o, ci) for ci in range(NB) for o in range(N_IT)]
                    else:
                        order = [(o, ci) for o in range(N_IT) for ci in range(NB)]
                    for o, ci in order:
                        nc.tensor.matmul(
                            accs[o][:],
                            v_sb[:, it * NB + ci, ts(o, 128)],
                            ft[:, ci, :],
                            start=(it == 0 and ci == 0),
                            stop=(it == N_IT - 1 and ci == NB - 1),
                        )
                # drain: add per-output bias; split across Scalar and Vector
                # engines so the last-slice tail is short.
                for o in range(N_IT):
                    ysb = ysb_pool.tile([128, BS], f32)
                    if o % 2 == 0:
                        nc.scalar.activation(
                            ysb[:], accs[o][:], AF.Identity,
                            bias=bias_sb[:, o : o + 1],
                        )
                    else:
                        nc.vector.tensor_scalar_add(
                            ysb[:], accs[o][:], bias_sb[:, o : o + 1]
                        )
                    # alternate output queues mid-kernel; the final slice goes
                    # all-sync (gpsimd wakeup + drain is slow at the very end)
                    dma_eng = nc.sync if (o % 2 == 0 or bs == N_BS - 1) else nc.gpsimd
                    dma_eng.dma_start(yT[ts(o, 128), ts(bs, BS)], ysb[:])

    nc.compile()
    _state["nc"] = nc
    return nc


def _silu_in_basis():
    """Project silu(x) on [-1.1, 1.1] onto the 8 B-spline bases, weighted by
    the clipped-N(0,1) input distribution (atoms at the clamp bounds)."""
    from math import erf, sqrt

    def n3(t):
        wp = np.maximum(np.minimum(t, 4 - t), 0.0)
        zp = np.maximum(np.minimum(t - 1, 3 - t), 0.0)
        return (wp**3 - 4 * zp**3) / 6.0

    x = np.linspace(-1.0999, 1.0999, 8001)
    w = np.exp(-x**2 / 2) / np.sqrt(2 * np.pi) * (x[1] - x[0])
    tail = 1 - 0.5 * (1 + erf(1.1 / sqrt(2)))
    X = np.concatenate([x, [-1.1, 1.1]])
    W = np.concatenate([w, [tail, tail]])
    s = 2.5 * X + 5.5
    Bm = np.stack([n3(s - g) for g in range(8)], axis=-1)
    F = X / (1 + np.exp(-X))
    swr = np.sqrt(W)
    c, *_ = np.linalg.lstsq(Bm * swr[:, None], F * swr, rcond=None)
    return c  # (8,) coefficients over normalized N3 bases


def _build_V(base_weight, spline_weight, spline_scaler):
    """Returns (V, bvec): the 28-chunk bf16 weight matrix and the f32 bias.

    Per (i,o), W_g = (sw*scaler)[o,i,g] + csil_g*bwT[i,o] are the weights
    over normalized bases N3(s-g).  Cubic p interpolates W at g in G_INT;
    blocks per input tile (order must match ft planes):
      0: tau    with weight c1 + c3/4
      1: tau^2  with weight c2
      2: tau^3  with weight c3
      3..6: 6*N3(s-g), g in G_RES, with weight (W_g - p(g))/6
    bias[o] = sum_i (c0 + c2/12).
    """
    sw = spline_weight.astype(np.float64) * spline_scaler.astype(np.float64)[:, :, None]
    W = np.transpose(sw, (2, 1, 0))  # (8, in, out)
    csil = _silu_in_basis()
    W = W + csil[:, None, None] * base_weight.astype(np.float64).T[None, :, :]

    Tm = lambda g, m: ((g - 3.5) / 2.0) ** m
    L = np.array([[Tm(g, m) for m in range(4)] for g in G_INT])
    Linv = np.linalg.inv(L)
    cw = np.einsum("ma,aio->mio", Linv, W[list(G_INT)])  # (4, in, out)

    bvec = (cw[0] + cw[2] / 12.0).sum(axis=0).astype(np.float32)  # (out,)

    blocks = [cw[1] + cw[3] / 4.0, cw[2], cw[3]]
    for g in G_RES:
        blocks.append((W[g] - sum(cw[m] * Tm(g, m) for m in range(4))) / 6.0)

    V = np.empty((128, KC, OUT_F), dtype=np.float32)
    for it in range(N_IT):
        isl = slice(it * 128, (it + 1) * 128)
        for j, blk in enumerate(blocks):
            V[:, it * NB + j, :] = blk[isl, :]
    import ml_dtypes
    bvec_h = np.ascontiguousarray(bvec.reshape(N_IT, 128).T)  # [p, t]
    return (
        np.ascontiguousarray(V.reshape(128, KC * OUT_F).astype(ml_dtypes.bfloat16)),
        bvec_h,
    )


def _make_in_maps(x, base_weight, spline_weight, spline_scaler):
    Vb, bvec = _build_V(base_weight, spline_weight, spline_scaler)
    x = np.asarray(x, dtype=np.float32)
    in_maps = []
    for c in range(N_CORES):
        xTc = np.ascontiguousarray(x[c * BPC : (c + 1) * BPC, :].T)
        in_maps.append({"xT": xTc, "V": Vb, "bvec": bvec})
    return in_maps


def kernel(x, base_weight, spline_weight, spline_scaler, grid):
    from concourse.bass_utils import run_bass_kernel_spmd

    nc = _build_kernel()
    in_maps = _make_in_maps(x, base_weight, spline_weight, spline_scaler)
    res = run_bass_kernel_spmd(nc, in_maps, core_ids=list(range(N_CORES)))
    y = np.empty((B, OUT_F), dtype=np.float32)
    for c in range(N_CORES):
        y[c * BPC : (c + 1) * BPC, :] = res.results[c]["yT"].T
    return y


# revision 30
# speedup vs baseline: 1.0103x; 1.0103x over previous
"""KANLinear forward on 8 Trainium2 NeuronCores (data-parallel over batch).

Factorization (v2: 7 K-blocks instead of 8)
-------------------------------------------
reference computes, per token row x (after clip/renorm preprocessing):
    y = silu(x) @ base_weight.T + einsum('big,oig->bo', bsplines(x), sw*scaler)

With s = 2.5*x + 5.5 the 8 cubic B-spline bases are B_g(x) = N3(s-g),
g = 0..7.  The silu path folds into the same basis (least-squares
projection), giving per (i,o) a weight vector W_g over the 8 bases.

v2 change-of-basis: interpolate a cubic p(g) = sum_m c_m*T_m(g),
T_m(g) = ((g-3.5)/2)^m, through W at g in {0,2,5,7}; the residual
r_g = W_g - p(g) is nonzero only at g in {1,3,4,6}.  Moment identities
for cubic B-splines (tau := (s-5.5)/2):
    sum_g T1(g) N3(s-g) = tau
    sum_g T2(g) N3(s-g) = tau^2 + 1/12
    sum_g T3(g) N3(s-g) = tau^3 + tau/4
so the polynomial part rides on cheap monomial features {tau, tau^2,
tau^3}, the constant parts collapse into a per-output bias added at
PSUM drain, and only 4 spline bases remain as ACT-table evaluations.
K shrinks 4096 -> 3584 (8 blocks -> 7) and the expensive per-element
spline evals drop 8 -> 4.  Edge defects (the g-sum truncation at
s<3 / s>8, where phantom bases g=-1/g=8 would be needed) contribute
rel err ~5e-3 (verified vs reference in fp64), within the 2e-2 gate.

Features 6*N3(s-g) are produced by ScalarE ACTIVATE through a custom
ACT table (the stock `sin` entry rewritten so activation(Sin,
scale=0.25, bias=(9.5-g)/8) returns 6*N3(s-g) exactly); tau powers by
two VectorE tensor_tensor ops off the KAN_PRE-preprocessed input.
Batch dim (16384) is sharded 2048 rows/core; weights are replicated.
"""

import hashlib
import os
import shutil
import tempfile

import numpy as np

B, IN_F, OUT_F = 16384, 512, 512
N_CORES = 8
BPC = B // N_CORES            # batch rows per core
BS = 512                      # batch-column slice processed per step
N_BS = BPC // BS              # 4 slices
N_IT = IN_F // 128            # 4 input-feature partition tiles
NB = 7                        # K-blocks per input tile (3 monomial + 4 spline)
KC = N_IT * NB                # 28 K-chunks of 128
G_INT = (0, 2, 5, 7)          # interpolation nodes (weights exactly absorbed)
G_RES = (1, 3, 4, 6)          # residual spline bases kept as ACT features
N_WARM_MM = 12                # dummy matmuls to ramp the PE clock at startup

_state = {}


# --------------------------------------------------------------------------
# Custom ACT table: hijack `sin` in silu_and_others to evaluate 6*N3(8u-4).
# Verified-on-HW stock mapping: ctrl entry = 42+(exp-116); entry 52 (binade
# [0.5,1)) has 8 sub-buckets of width 1/16 at buckets 1034..1041; bucket
# eval is y = d0+(u-x0)(d1+(u-x0)(d2+(u-x0)d3)); |u|<2^-11 -> bucket
# 1075/1076 (sign-folded); large |u| -> 1077/1078.  Buckets 1020..1078 are
# sin-private; everything else (silu, copy, ...) is untouched.
# --------------------------------------------------------------------------
def _n3_6_coeffs(j):
    return {
        0: [0.0, 0.0, 0.0, 1.0],
        1: [1.0, 3.0, 3.0, -3.0],
        2: [4.0, 0.0, -6.0, 3.0],
        3: [1.0, -3.0, 3.0, -1.0],
    }[j]


def _compose(c, scale, shift):
    c0, c1, c2, c3 = c
    return [
        c0 + c1 * shift + c2 * shift**2 + c3 * shift**3,
        scale * (c1 + 2 * c2 * shift + 3 * c3 * shift**2),
        scale**2 * (c2 + 3 * c3 * shift),
        scale**3 * c3,
    ]


def _build_custom_act_root():
    if "act_root" in _state:
        return _state["act_root"], _state["act_sig"]
    from neuronxcc.driver.Job import Job
    from neuronxcc.driver.jobs.support.FindActInfo import findActInfoFile

    src_json = findActInfoFile(Job.getPackageDir(), "gen3")
    src_dir = os.path.dirname(src_json)
    dst_dir = tempfile.mkdtemp(prefix="kan_act_root_")
    for f in os.listdir(src_dir):
        shutil.copy(os.path.join(src_dir, f), os.path.join(dst_dir, f))
    for f in os.listdir(dst_dir):
        os.chmod(os.path.join(dst_dir, f), 0o644)

    bkt_path = os.path.join(dst_dir, "silu_and_others_bkt.bin")
    bkt = np.fromfile(bkt_path, dtype=np.float32).reshape(-1, 8).copy()
    bkt[1020:1079] = 0.0
    for k in range(8):
        x0 = 0.5 + k / 16.0 + 1.0 / 32.0
        j = k // 2
        q = _compose(_n3_6_coeffs(j), 8.0, 8.0 * x0 - 4.0 - j)
        bkt[1034 + k] = [q[0], q[1], q[2], q[3], x0, 0.0, 0.0, 0.0]
    bkt.tofile(bkt_path)

    sig = hashlib.sha256(open(bkt_path, "rb").read()).hexdigest()[:10]
    path = os.path.join(dst_dir, "act_info.json")
    os.environ["BASS_ACT_ROOT_JSON_PATH"] = path
    _state["act_root"] = path
    _state["act_sig"] = sig
    return path, sig


# --------------------------------------------------------------------------
# Custom DVE op: preprocessing clip(x,-1.1,1.1)*1.25 -> tau = (s-5.5)/2
# --------------------------------------------------------------------------
def _register_ops():
    if "ops" in _state:
        return _state["ops"]
    import concourse.dve_ops as dve_ops
    from concourse.dve_spec import Spec, Src0, C0, C1, C2, One, maxx, minn, lower
    from concourse.dve_uop import DveOpSpec

    def pre_ref(in0, in1, s0, s1, imm2):
        t = np.minimum(np.maximum(in0, np.float32(s0)), np.float32(s1))
        t = ((t + np.float32(1)) - np.float32(1)).astype(np.float32)
        return (t * np.float32(imm2)).astype(np.float32)

    pre_spec = Spec(
        body=((minn(maxx(Src0, C0), C1) + One) - One) * C2, reference=pre_ref
    )

    ops = {}
    name = "KAN_PRE"
    if name in dve_ops._SUB_OPCODE_FOR_NAME:
        ops[name] = next(o for o in dve_ops.OPS if o.name == name)
    else:
        row = dve_ops._CUSTOM_DVE_ROW_BASE + len(dve_ops.OPS)
        assert row < 0x20, "custom-DVE row overflow"
        shas = {}
        for ver in ("v3", "v4"):
            try:
                tmp = DveOpSpec(
                    name=name, opcode=row, uops=lower(pre_spec, ver=ver),
                    rd1_en=dve_ops.has_src1(pre_spec),
                )
                shas[ver] = tmp.sha(ver)
            except Exception:
                pass
        op = dve_ops.DveOp(name, pre_spec, subdim=False, uops_sha=shas)
        dve_ops.OPS.append(op)
        dve_ops._SUB_OPCODE_FOR_NAME[name] = row
        dve_ops.CUSTOM_DVE_SPECS[name] = pre_spec
        ops[name] = op
    _state["ops"] = ops
    return ops


# --------------------------------------------------------------------------
# Kernel build
# --------------------------------------------------------------------------
def _build_kernel():
    if "nc" in _state:
        return _state["nc"]
    import concourse.bacc as bacc
    import concourse.mybir as mybir
    import concourse.tile as tile
    from concourse.bass import ts

    _, act_sig = _build_custom_act_root()
    ops = _register_ops()
    f32 = mybir.dt.float32
    bf16 = mybir.dt.bfloat16
    AF = mybir.ActivationFunctionType
    ALU = mybir.AluOpType

    nc = bacc.Bacc()
    # Register const APs for the per-basis ACT biases.  The act-table
    # signature is baked into the tensor name so NEFF caches can never mix
    # incompatible act tables with this BIR.
    for g in G_RES:
        val = (9.5 - g) / 8.0
        t = nc.alloc_sbuf_tensor(f"cbias{g}-{act_sig}", [128, 1], f32)
        nc.gpsimd.memset(t.ap(), val)
        nc.const_aps.aps[(f32, val)] = t.ap()
    nc.all_engine_barrier()

    xT = nc.dram_tensor("xT", [IN_F, BPC], f32, kind="ExternalInput")
    # V is laid out partition-major on the host ([sbuf partition, chunk, out])
    # so each per-partition DMA run is one contiguous 28KB read.
    V = nc.dram_tensor("V", [128, KC * OUT_F], bf16, kind="ExternalInput")
    # host-permuted: bvec[p, t] = bias[t*128 + p] so the DMA is contiguous
    bvec = nc.dram_tensor("bvec", [128, N_IT], f32, kind="ExternalInput")
    yT = nc.dram_tensor("yT", [OUT_F, BPC], f32, kind="ExternalOutput")

    with tile.TileContext(nc) as tc:
        with (
            tc.tile_pool(name="vpool", bufs=1) as vpool,
            tc.tile_pool(name="warmp", bufs=1) as warm_pool,
            tc.tile_pool(name="xin", bufs=6) as xin_pool,
            tc.tile_pool(name="xs", bufs=4) as xs_pool,
            tc.tile_pool(name="feat", bufs=8) as feat_pool,
            tc.tile_pool(name="ysb", bufs=6) as ysb_pool,
            tc.tile_pool(name="psum", bufs=8, space="PSUM") as psum_pool,
        ):
            # Weight DMAs first on the gpsimd queue so the transfers start
            # the moment the engine exits the NEFF preamble.
            v_sb = vpool.tile([128, KC, OUT_F], bf16)
            v_view = V[:].rearrange("p (kc o) -> p kc o", o=OUT_F)
            # it0 alone first so its completion semaphore posts the moment it
            # streams (swdge finishes a trigger fully before the next one);
            # the remaining tiles ride one big second trigger.
            # it0 arrives in two pieces: the monomial planes (0-2) land first
            # so the earliest ci-major matmuls can start ~1.5us sooner.
            nc.scalar.dma_start(v_sb[:, 0:3, :], v_view[:, 0:3, :])
            nc.gpsimd.dma_start(v_sb[:, 3:NB, :], v_view[:, 3:NB, :])
            bias_sb = vpool.tile([128, N_IT], f32)
            nc.gpsimd.dma_start(bias_sb[:], bvec[:])
            # bs0's x tiles are queued on sync BEFORE the remaining weight
            # tiles so the first feature chain isn't starved for bandwidth;
            # V-it1..3 follow on the same ring and still land with margin.
            xins0 = []
            for it in range(N_IT):
                xin = xin_pool.tile([128, BS], f32)
                nc.sync.dma_start(xin[:], xT[ts(it, 128), 0:BS])
                xins0.append(xin)
            for it in range(1, N_IT):
                nc.sync.dma_start(
                    v_sb[:, ts(it, NB), :], v_view[:, ts(it, NB), :]
                )

            # Kick the ACT table load for silu_and_others immediately so it
            # overlaps the first input DMA instead of the first feature chain.
            warm = xs_pool.tile([128, 1], f32, name="warm", tag="warm")
            nc.vector.memset(warm[:], 0.0)
            nc.scalar.activation(warm[:], warm[:], AF.Silu)

            # PE clock-ramp dummies: zero matmuls with no DMA dependency run
            # while the weight/input DMAs are in flight, so the real matmuls
            # start at full clock.
            wstat = warm_pool.tile([128, 128], bf16, name="wstat")
            wmov = warm_pool.tile([128, BS], bf16, name="wmov")
            nc.vector.memset(wstat[:], 0.0)
            nc.vector.memset(wmov[:], 0.0)
            wacc = psum_pool.tile([128, BS], f32, name="wacc", tag="acc")
            for _ in range(N_WARM_MM):
                nc.tensor.matmul(wacc[:], wstat[:], wmov[:], start=True, stop=True)



            for bs in range(N_BS):
                accs = [
                    psum_pool.tile([128, BS], f32, name=f"acc{o}", tag="acc")
                    for o in range(N_IT)
                ]
                for it in range(N_IT):
                    if bs == 0:
                        xin = xins0[it]
                    else:
                        xin = xin_pool.tile([128, BS], f32)
                        nc.sync.dma_start(xin[:], xT[ts(it, 128), ts(bs, BS)])
                    # tau = clip(x,-1.1,1.1)*1.25 = (s-5.5)/2, fp32
                    xs = xs_pool.tile([128, BS], f32)
                    nc.vector._custom_dve(
                        ops["KAN_PRE"], out=xs[:], in0=xin[:],
                        s0=-1.1, s1=1.1, imm2=1.25,
                    )
                    ft = feat_pool.tile([128, NB, BS], bf16)
                    # monomial features: tau, tau^2, tau^3 (VectorE)
                    nc.vector.tensor_scalar_mul(ft[:, 0, :], xs[:], 1.0)
                    nc.vector.tensor_tensor(
                        ft[:, 1, :], xs[:], xs[:], ALU.mult
                    )
                    nc.vector.tensor_tensor(
                        ft[:, 2, :], ft[:, 1, :], xs[:], ALU.mult
                    )
                    # residual spline bases 6*N3(s-g) via custom ACT table
                    for k, g in enumerate(G_RES):
                        nc.scalar.activation(
                            ft[:, 3 + k, :], xs[:], AF.Sin,
                            scale=0.25, bias=(9.5 - g) / 8.0,
                        )
                    # ci-major on the very first tile so the earliest matmuls
                    # depend only on the fast DVE monomial planes while the
                    # ACT-table bases are still loading; o-major elsewhere so
                    # drains stagger at the end of each batch slice.
                    if bs == 0 and it == 0:
                        order = [(o, ci) for ci in range(NB) for o in range(N_IT)]
                    else:
                        order = [(o, ci) for o in range(N_IT) for ci in range(NB)]
                    for o, ci in order:
                        nc.tensor.matmul(
                            accs[o][:],
                            v_sb[:, it * NB + ci, ts(o, 128)],
                            ft[:, ci, :],
                            start=(it == 0 and ci == 0),
                            stop=(it == N_IT - 1 and ci == NB - 1),
                        )
                # drain: add per-output bias; split across Scalar and Vector
                # engines so the last-slice tail is short.
                for o in range(N_IT):
                    ysb = ysb_pool.tile([128, BS], f32)
                    if o % 2 == 0:
                        nc.scalar.activation(
                            ysb[:], accs[o][:], AF.Identity,
                            bias=bias_sb[:, o : o + 1],
                        )
                    else:
                        nc.vector.tensor_scalar_add(
                            ysb[:], accs[o][:], bias_sb[:, o : o + 1]
                        )
                    # alternate output queues mid-kernel; the final slice goes
                    # all-sync (gpsimd wakeup + drain is slow at the very end)
                    dma_eng = nc.sync if (o % 2 == 0 or bs == N_BS - 1) else nc.gpsimd
                    dma_eng.dma_start(yT[ts(o, 128), ts(bs, BS)], ysb[:])

    nc.compile()
    _state["nc"] = nc
    return nc


def _silu_in_basis():
    """Project silu(x) on [-1.1, 1.1] onto the 8 B-spline bases, weighted by
    the clipped-N(0,1) input distribution (atoms at the clamp bounds)."""
    from math import erf, sqrt

    def n3(t):
        wp = np.maximum(np.minimum(t, 4 - t), 0.0)
        zp = np.maximum(np.minimum(t - 1, 3 - t), 0.0)
        return (wp**3 - 4 * zp**3) / 6.0

    x = np.linspace(-1.0999, 1.0999, 8001)
    w = np.exp(-x**2 / 2) / np.sqrt(2 * np.pi) * (x[1] - x[0])
    tail = 1 - 0.5 * (1 + erf(1.1 / sqrt(2)))
    X = np.concatenate([x, [-1.1, 1.1]])
    W = np.concatenate([w, [tail, tail]])
    s = 2.5 * X + 5.5
    Bm = np.stack([n3(s - g) for g in range(8)], axis=-1)
    F = X / (1 + np.exp(-X))
    swr = np.sqrt(W)
    c, *_ = np.linalg.lstsq(Bm * swr[:, None], F * swr, rcond=None)
    return c  # (8,) coefficients over normalized N3 bases


def _build_V(base_weight, spline_weight, spline_scaler):
    """Returns (V, bvec): the 28-chunk bf16 weight matrix and the f32 bias.

    Per (i,o), W_g = (sw*scaler)[o,i,g] + csil_g*bwT[i,o] are the weights
    over normalized bases N3(s-g).  Cubic p interpolates W at g in G_INT;
    blocks per input tile (order must match ft planes):
      0: tau    with weight c1 + c3/4
      1: tau^2  with weight c2
      2: tau^3  with weight c3
      3..6: 6*N3(s-g), g in G_RES, with weight (W_g - p(g))/6
    bias[o] = sum_i (c0 + c2/12).
    """
    sw = spline_weight.astype(np.float64) * spline_scaler.astype(np.float64)[:, :, None]
    W = np.transpose(sw, (2, 1, 0))  # (8, in, out)
    csil = _silu_in_basis()
    W = W + csil[:, None, None] * base_weight.astype(np.float64).T[None, :, :]

    Tm = lambda g, m: ((g - 3.5) / 2.0) ** m
    L = np.array([[Tm(g, m) for m in range(4)] for g in G_INT])
    Linv = np.linalg.inv(L)
    cw = np.einsum("ma,aio->mio", Linv, W[list(G_INT)])  # (4, in, out)

    bvec = (cw[0] + cw[2] / 12.0).sum(axis=0).astype(np.float32)  # (out,)

    blocks = [cw[1] + cw[3] / 4.0, cw[2], cw[3]]
    for g in G_RES:
        blocks.append((W[g] - sum(cw[m] * Tm(g, m) for m in range(4))) / 6.0)

    V = np.empty((128, KC, OUT_F), dtype=np.float32)
    for it in range(N_IT):
        isl = slice(it * 128, (it + 1) * 128)
        for j, blk in enumerate(blocks):
            V[:, it * NB + j, :] = blk[isl, :]
    import ml_dtypes
    bvec_h = np.ascontiguousarray(bvec.reshape(N_IT, 128).T)  # [p, t]
    return (
        np.ascontiguousarray(V.reshape(128, KC * OUT_F).astype(ml_dtypes.bfloat16)),
        bvec_h,
    )


def _make_in_maps(x, base_weight, spline_weight, spline_scaler):
    Vb, bvec = _build_V(base_weight, spline_weight, spline_scaler)
    x = np.asarray(x, dtype=np.float32)
    in_maps = []
    for c in range(N_CORES):
        xTc = np.ascontiguousarray(x[c * BPC : (c + 1) * BPC, :].T)
        in_maps.append({"xT": xTc, "V": Vb, "bvec": bvec})
    return in_maps


def kernel(x, base_weight, spline_weight, spline_scaler, grid):
    from concourse.bass_utils import run_bass_kernel_spmd

    nc = _build_kernel()
    in_maps = _make_in_maps(x, base_weight, spline_weight, spline_scaler)
    res = run_bass_kernel_spmd(nc, in_maps, core_ids=list(range(N_CORES)))
    y = np.empty((B, OUT_F), dtype=np.float32)
    for c in range(N_CORES):
        y[c * BPC : (c + 1) * BPC, :] = res.results[c]["yT"].T
    return y


# revision 31
# speedup vs baseline: 1.0225x; 1.0121x over previous
"""KANLinear forward on 8 Trainium2 NeuronCores (data-parallel over batch).

Factorization (v2: 7 K-blocks instead of 8)
-------------------------------------------
reference computes, per token row x (after clip/renorm preprocessing):
    y = silu(x) @ base_weight.T + einsum('big,oig->bo', bsplines(x), sw*scaler)

With s = 2.5*x + 5.5 the 8 cubic B-spline bases are B_g(x) = N3(s-g),
g = 0..7.  The silu path folds into the same basis (least-squares
projection), giving per (i,o) a weight vector W_g over the 8 bases.

v2 change-of-basis: interpolate a cubic p(g) = sum_m c_m*T_m(g),
T_m(g) = ((g-3.5)/2)^m, through W at g in {0,2,5,7}; the residual
r_g = W_g - p(g) is nonzero only at g in {1,3,4,6}.  Moment identities
for cubic B-splines (tau := (s-5.5)/2):
    sum_g T1(g) N3(s-g) = tau
    sum_g T2(g) N3(s-g) = tau^2 + 1/12
    sum_g T3(g) N3(s-g) = tau^3 + tau/4
so the polynomial part rides on cheap monomial features {tau, tau^2,
tau^3}, the constant parts collapse into a per-output bias added at
PSUM drain, and only 4 spline bases remain as ACT-table evaluations.
K shrinks 4096 -> 3584 (8 blocks -> 7) and the expensive per-element
spline evals drop 8 -> 4.  Edge defects (the g-sum truncation at
s<3 / s>8, where phantom bases g=-1/g=8 would be needed) contribute
rel err ~5e-3 (verified vs reference in fp64), within the 2e-2 gate.

Features 6*N3(s-g) are produced by ScalarE ACTIVATE through a custom
ACT table (the stock `sin` entry rewritten so activation(Sin,
scale=0.25, bias=(9.5-g)/8) returns 6*N3(s-g) exactly); tau powers by
two VectorE tensor_tensor ops off the KAN_PRE-preprocessed input.
Batch dim (16384) is sharded 2048 rows/core; weights are replicated.
"""

import hashlib
import os
import shutil
import tempfile

import numpy as np

B, IN_F, OUT_F = 16384, 512, 512
N_CORES = 8
BPC = B // N_CORES            # batch rows per core
BS = 512                      # batch-column slice processed per step
N_BS = BPC // BS              # 4 slices
N_IT = IN_F // 128            # 4 input-feature partition tiles
NB = 7                        # K-blocks per input tile (3 monomial + 4 spline)
KC = N_IT * NB                # 28 K-chunks of 128
G_INT = (0, 2, 5, 7)          # interpolation nodes (weights exactly absorbed)
G_RES = (1, 3, 4, 6)          # residual spline bases kept as ACT features
N_WARM_MM = 12                # dummy matmuls to ramp the PE clock at startup

_state = {}


# --------------------------------------------------------------------------
# Custom ACT table: hijack `sin` in silu_and_others to evaluate 6*N3(8u-4).
# Verified-on-HW stock mapping: ctrl entry = 42+(exp-116); entry 52 (binade
# [0.5,1)) has 8 sub-buckets of width 1/16 at buckets 1034..1041; bucket
# eval is y = d0+(u-x0)(d1+(u-x0)(d2+(u-x0)d3)); |u|<2^-11 -> bucket
# 1075/1076 (sign-folded); large |u| -> 1077/1078.  Buckets 1020..1078 are
# sin-private; everything else (silu, copy, ...) is untouched.
# --------------------------------------------------------------------------
def _n3_6_coeffs(j):
    return {
        0: [0.0, 0.0, 0.0, 1.0],
        1: [1.0, 3.0, 3.0, -3.0],
        2: [4.0, 0.0, -6.0, 3.0],
        3: [1.0, -3.0, 3.0, -1.0],
    }[j]


def _compose(c, scale, shift):
    c0, c1, c2, c3 = c
    return [
        c0 + c1 * shift + c2 * shift**2 + c3 * shift**3,
        scale * (c1 + 2 * c2 * shift + 3 * c3 * shift**2),
        scale**2 * (c2 + 3 * c3 * shift),
        scale**3 * c3,
    ]


def _build_custom_act_root():
    if "act_root" in _state:
        return _state["act_root"], _state["act_sig"]
    from neuronxcc.driver.Job import Job
    from neuronxcc.driver.jobs.support.FindActInfo import findActInfoFile

    src_json = findActInfoFile(Job.getPackageDir(), "gen3")
    src_dir = os.path.dirname(src_json)
    dst_dir = tempfile.mkdtemp(prefix="kan_act_root_")
    for f in os.listdir(src_dir):
        shutil.copy(os.path.join(src_dir, f), os.path.join(dst_dir, f))
    for f in os.listdir(dst_dir):
        os.chmod(os.path.join(dst_dir, f), 0o644)

    bkt_path = os.path.join(dst_dir, "silu_and_others_bkt.bin")
    bkt = np.fromfile(bkt_path, dtype=np.float32).reshape(-1, 8).copy()
    bkt[1020:1079] = 0.0
    for k in range(8):
        x0 = 0.5 + k / 16.0 + 1.0 / 32.0
        j = k // 2
        q = _compose(_n3_6_coeffs(j), 8.0, 8.0 * x0 - 4.0 - j)
        bkt[1034 + k] = [q[0], q[1], q[2], q[3], x0, 0.0, 0.0, 0.0]
    bkt.tofile(bkt_path)

    sig = hashlib.sha256(open(bkt_path, "rb").read()).hexdigest()[:10]
    path = os.path.join(dst_dir, "act_info.json")
    os.environ["BASS_ACT_ROOT_JSON_PATH"] = path
    _state["act_root"] = path
    _state["act_sig"] = sig
    return path, sig


# --------------------------------------------------------------------------
# Custom DVE op: preprocessing clip(x,-1.1,1.1)*1.25 -> tau = (s-5.5)/2
# --------------------------------------------------------------------------
def _register_ops():
    if "ops" in _state:
        return _state["ops"]
    import concourse.dve_ops as dve_ops
    from concourse.dve_spec import Spec, Src0, C0, C1, C2, One, maxx, minn, lower
    from concourse.dve_uop import DveOpSpec

    def pre_ref(in0, in1, s0, s1, imm2):
        t = np.minimum(np.maximum(in0, np.float32(s0)), np.float32(s1))
        t = ((t + np.float32(1)) - np.float32(1)).astype(np.float32)
        return (t * np.float32(imm2)).astype(np.float32)

    pre_spec = Spec(
        body=((minn(maxx(Src0, C0), C1) + One) - One) * C2, reference=pre_ref
    )

    ops = {}
    name = "KAN_PRE"
    if name in dve_ops._SUB_OPCODE_FOR_NAME:
        ops[name] = next(o for o in dve_ops.OPS if o.name == name)
    else:
        row = dve_ops._CUSTOM_DVE_ROW_BASE + len(dve_ops.OPS)
        assert row < 0x20, "custom-DVE row overflow"
        shas = {}
        for ver in ("v3", "v4"):
            try:
                tmp = DveOpSpec(
                    name=name, opcode=row, uops=lower(pre_spec, ver=ver),
                    rd1_en=dve_ops.has_src1(pre_spec),
                )
                shas[ver] = tmp.sha(ver)
            except Exception:
                pass
        op = dve_ops.DveOp(name, pre_spec, subdim=False, uops_sha=shas)
        dve_ops.OPS.append(op)
        dve_ops._SUB_OPCODE_FOR_NAME[name] = row
        dve_ops.CUSTOM_DVE_SPECS[name] = pre_spec
        ops[name] = op
    _state["ops"] = ops
    return ops


# --------------------------------------------------------------------------
# Kernel build
# --------------------------------------------------------------------------
def _build_kernel():
    if "nc" in _state:
        return _state["nc"]
    import concourse.bacc as bacc
    import concourse.mybir as mybir
    import concourse.tile as tile
    from concourse.bass import ts

    _, act_sig = _build_custom_act_root()
    ops = _register_ops()
    f32 = mybir.dt.float32
    bf16 = mybir.dt.bfloat16
    AF = mybir.ActivationFunctionType
    ALU = mybir.AluOpType

    nc = bacc.Bacc()
    # Register const APs for the per-basis ACT biases.  The act-table
    # signature is baked into the tensor name so NEFF caches can never mix
    # incompatible act tables with this BIR.
    for g in G_RES:
        val = (9.5 - g) / 8.0
        t = nc.alloc_sbuf_tensor(f"cbias{g}-{act_sig}", [128, 1], f32)
        nc.gpsimd.memset(t.ap(), val)
        nc.const_aps.aps[(f32, val)] = t.ap()
    nc.all_engine_barrier()

    xT = nc.dram_tensor("xT", [IN_F, BPC], f32, kind="ExternalInput")
    # V is laid out partition-major on the host ([sbuf partition, chunk, out])
    # so each per-partition DMA run is one contiguous 28KB read.
    V = nc.dram_tensor("V", [128, KC * OUT_F], bf16, kind="ExternalInput")
    # host-permuted: bvec[p, t] = bias[t*128 + p] so the DMA is contiguous
    bvec = nc.dram_tensor("bvec", [128, N_IT], f32, kind="ExternalInput")
    yT = nc.dram_tensor("yT", [OUT_F, BPC], f32, kind="ExternalOutput")

    with tile.TileContext(nc) as tc:
        with (
            tc.tile_pool(name="vpool", bufs=1) as vpool,
            tc.tile_pool(name="warmp", bufs=1) as warm_pool,
            tc.tile_pool(name="xin", bufs=6) as xin_pool,
            tc.tile_pool(name="xs", bufs=4) as xs_pool,
            tc.tile_pool(name="feat", bufs=8) as feat_pool,
            tc.tile_pool(name="ysb", bufs=6) as ysb_pool,
            tc.tile_pool(name="psum", bufs=8, space="PSUM") as psum_pool,
        ):
            # Weight DMAs first on the gpsimd queue so the transfers start
            # the moment the engine exits the NEFF preamble.
            v_sb = vpool.tile([128, KC, OUT_F], bf16)
            v_view = V[:].rearrange("p (kc o) -> p kc o", o=OUT_F)
            # it0 alone first so its completion semaphore posts the moment it
            # streams (swdge finishes a trigger fully before the next one);
            # the remaining tiles ride one big second trigger.
            # it0 arrives in two pieces: the monomial planes (0-2) land first
            # so the earliest ci-major matmuls can start ~1.5us sooner.
            nc.gpsimd.dma_start(v_sb[:, 0:3, :], v_view[:, 0:3, :])
            nc.gpsimd.dma_start(v_sb[:, 3:NB, :], v_view[:, 3:NB, :])
            bias_sb = vpool.tile([128, N_IT], f32)
            nc.gpsimd.dma_start(bias_sb[:], bvec[:])
            # bs0's x tiles are queued on sync BEFORE the remaining weight
            # tiles so the first feature chain isn't starved for bandwidth;
            # V-it1..3 follow on the same ring and still land with margin.
            xins0 = []
            for it in range(N_IT):
                xin = xin_pool.tile([128, BS], f32)
                nc.sync.dma_start(xin[:], xT[ts(it, 128), 0:BS])
                xins0.append(xin)
            for it in range(1, N_IT):
                nc.sync.dma_start(
                    v_sb[:, ts(it, NB), :], v_view[:, ts(it, NB), :]
                )

            # Kick the ACT table load for silu_and_others immediately so it
            # overlaps the first input DMA instead of the first feature chain.
            warm = xs_pool.tile([128, 1], f32, name="warm", tag="warm")
            nc.vector.memset(warm[:], 0.0)
            nc.scalar.activation(warm[:], warm[:], AF.Silu)

            # PE clock-ramp dummies: zero matmuls with no DMA dependency run
            # while the weight/input DMAs are in flight, so the real matmuls
            # start at full clock.
            wstat = warm_pool.tile([128, 128], bf16, name="wstat")
            wmov = warm_pool.tile([128, BS], bf16, name="wmov")
            nc.vector.memset(wstat[:], 0.0)
            nc.vector.memset(wmov[:], 0.0)
            wacc = psum_pool.tile([128, BS], f32, name="wacc", tag="acc")
            for _ in range(N_WARM_MM):
                nc.tensor.matmul(wacc[:], wstat[:], wmov[:], start=True, stop=True)



            for bs in range(N_BS):
                accs = [
                    psum_pool.tile([128, BS], f32, name=f"acc{o}", tag="acc")
                    for o in range(N_IT)
                ]
                for it in range(N_IT):
                    if bs == 0:
                        xin = xins0[it]
                    else:
                        xin = xin_pool.tile([128, BS], f32)
                        nc.sync.dma_start(xin[:], xT[ts(it, 128), ts(bs, BS)])
                    # tau = clip(x,-1.1,1.1)*1.25 = (s-5.5)/2, fp32
                    xs = xs_pool.tile([128, BS], f32)
                    nc.vector._custom_dve(
                        ops["KAN_PRE"], out=xs[:], in0=xin[:],
                        s0=-1.1, s1=1.1, imm2=1.25,
                    )
                    ft = feat_pool.tile([128, NB, BS], bf16)
                    # monomial features: tau, tau^2, tau^3 (VectorE)
                    nc.vector.tensor_scalar_mul(ft[:, 0, :], xs[:], 1.0)
                    nc.vector.tensor_tensor(
                        ft[:, 1, :], xs[:], xs[:], ALU.mult
                    )
                    nc.vector.tensor_tensor(
                        ft[:, 2, :], ft[:, 1, :], xs[:], ALU.mult
                    )
                    # residual spline bases 6*N3(s-g) via custom ACT table
                    for k, g in enumerate(G_RES):
                        nc.scalar.activation(
                            ft[:, 3 + k, :], xs[:], AF.Sin,
                            scale=0.25, bias=(9.5 - g) / 8.0,
                        )
                    # ci-major on the very first tile so the earliest matmuls
                    # depend only on the fast DVE monomial planes while the
                    # ACT-table bases are still loading; o-major elsewhere so
                    # drains stagger at the end of each batch slice.
                    if bs == 0 and it == 0:
                        order = [(o, ci) for ci in range(NB) for o in range(N_IT)]
                    else:
                        order = [(o, ci) for o in range(N_IT) for ci in range(NB)]
                    for o, ci in order:
                        nc.tensor.matmul(
                            accs[o][:],
                            v_sb[:, it * NB + ci, ts(o, 128)],
                            ft[:, ci, :],
                            start=(it == 0 and ci == 0),
                            stop=(it == N_IT - 1 and ci == NB - 1),
                        )
                # drain: add per-output bias; split across Scalar and Vector
                # engines so the last-slice tail is short.
                for o in range(N_IT):
                    ysb = ysb_pool.tile([128, BS], f32)
                    if o % 2 == 0:
                        nc.scalar.activation(
                            ysb[:], accs[o][:], AF.Identity,
                            bias=bias_sb[:, o : o + 1],
                        )
                    else:
                        nc.vector.tensor_scalar_add(
                            ysb[:], accs[o][:], bias_sb[:, o : o + 1]
                        )
                    # alternate output queues mid-kernel; the final slice goes
                    # all-sync (gpsimd wakeup + drain is slow at the very end)
                    dma_eng = nc.sync if (o % 2 == 0 or bs == N_BS - 1) else nc.gpsimd
                    dma_eng.dma_start(yT[ts(o, 128), ts(bs, BS)], ysb[:])

    nc.compile()
    _state["nc"] = nc
    return nc


def _silu_in_basis():
    """Project silu(x) on [-1.1, 1.1] onto the 8 B-spline bases, weighted by
    the clipped-N(0,1) input distribution (atoms at the clamp bounds)."""
    from math import erf, sqrt

    def n3(t):
        wp = np.maximum(np.minimum(t, 4 - t), 0.0)
        zp = np.maximum(np.minimum(t - 1, 3 - t), 0.0)
        return (wp**3 - 4 * zp**3) / 6.0

    x = np.linspace(-1.0999, 1.0999, 8001)
    w = np.exp(-x**2 / 2) / np.sqrt(2 * np.pi) * (x[1] - x[0])
    tail = 1 - 0.5 * (1 + erf(1.1 / sqrt(2)))
    X = np.concatenate([x, [-1.1, 1.1]])
    W = np.concatenate([w, [tail, tail]])
    s = 2.5 * X + 5.5
    Bm = np.stack([n3(s - g) for g in range(8)], axis=-1)
    F = X / (1 + np.exp(-X))
    swr = np.sqrt(W)
    c, *_ = np.linalg.lstsq(Bm * swr[:, None], F * swr, rcond=None)
    return c  # (8,) coefficients over normalized N3 bases


def _build_V(base_weight, spline_weight, spline_scaler):
    """Returns (V, bvec): the 28-chunk bf16 weight matrix and the f32 bias.

    Per (i,o), W_g = (sw*scaler)[o,i,g] + csil_g*bwT[i,o] are the weights
    over normalized bases N3(s-g).  Cubic p interpolates W at g in G_INT;
    blocks per input tile (order must match ft planes):
      0: tau    with weight c1 + c3/4
      1: tau^2  with weight c2
      2: tau^3  with weight c3
      3..6: 6*N3(s-g), g in G_RES, with weight (W_g - p(g))/6
    bias[o] = sum_i (c0 + c2/12).
    """
    sw = spline_weight.astype(np.float64) * spline_scaler.astype(np.float64)[:, :, None]
    W = np.transpose(sw, (2, 1, 0))  # (8, in, out)
    csil = _silu_in_basis()
    W = W + csil[:, None, None] * base_weight.astype(np.float64).T[None, :, :]

    Tm = lambda g, m: ((g - 3.5) / 2.0) ** m
    L = np.array([[Tm(g, m) for m in range(4)] for g in G_INT])
    Linv = np.linalg.inv(L)
    cw = np.einsum("ma,aio->mio", Linv, W[list(G_INT)])  # (4, in, out)

    bvec = (cw[0] + cw[2] / 12.0).sum(axis=0).astype(np.float32)  # (out,)

    blocks = [cw[1] + cw[3] / 4.0, cw[2], cw[3]]
    for g in G_RES:
        blocks.append((W[g] - sum(cw[m] * Tm(g, m) for m in range(4))) / 6.0)

    V = np.empty((128, KC, OUT_F), dtype=np.float32)
    for it in range(N_IT):
        isl = slice(it * 128, (it + 1) * 128)
        for j, blk in enumerate(blocks):
            V[:, it * NB + j, :] = blk[isl, :]
    import ml_dtypes
    bvec_h = np.ascontiguousarray(bvec.reshape(N_IT, 128).T)  # [p, t]
    return (
        np.ascontiguousarray(V.reshape(128, KC * OUT_F).astype(ml_dtypes.bfloat16)),
        bvec_h,
    )


def _make_in_maps(x, base_weight, spline_weight, spline_scaler):
    Vb, bvec = _build_V(base_weight, spline_weight, spline_scaler)
    x = np.asarray(x, dtype=np.float32)
    in_maps = []
    for c in range(N_CORES):
        xTc = np.ascontiguousarray(x[c * BPC : (c + 1) * BPC, :].T)
        in_maps.append({"xT": xTc, "V": Vb, "bvec": bvec})
    return in_maps


def kernel(x, base_weight, spline_weight, spline_scaler, grid):
    from concourse.bass_utils import run_bass_kernel_spmd

    nc = _build_kernel()
    in_maps = _make_in_maps(x, base_weight, spline_weight, spline_scaler)
    res = run_bass_kernel_spmd(nc, in_maps, core_ids=list(range(N_CORES)))
    y = np.empty((B, OUT_F), dtype=np.float32)
    for c in range(N_CORES):
        y[c * BPC : (c + 1) * BPC, :] = res.results[c]["yT"].T
    return y
